# revision 21
# baseline (speedup 1.0000x reference)
"""GEMM + reduce-scatter (nn_GemmRSIntraNode) as a Bass/Tile kernel on 8 trn2 cores.

Full semantics: out = einsum('rmk,rnk->mn', input, weight).reshape(8, 1024, 4096)
with input [8, 8192, 1024] f32 and weight [8, 4096, 1024] f32.

Sharding choice: instead of mimicking the per-rank partial-GEMM +
reduce-scatter, each core c directly computes output rows
[c*1024:(c+1)*1024] of the reduced result:

    out_c = sum_{r,k} input[r, c*1024:(c+1)*1024, k] * weight[r, n, k]

i.e. a [1024, 8192] x [8192, 4096] GEMM per core where the contraction
axis is (r, k) flattened. The rank-sum IS the K-axis contraction, so no
cross-core communication is needed at all; the "reduce-scatter" is
absorbed into the GEMM. Inputs are pre-transposed host-side into
[K, M] / [K, N] layouts so the device kernel does only contiguous DMA
and matmuls.

Device schedule (v2): compute in bf16 (norm rel err ~2.3e-3, well under
the 2e-2 gate; same PE rate as fp32r but half the HBM traffic). The
whole A slice [K, 1024] bf16 = 16 MB stays resident in SBUF, so B
streams through exactly once (64 MB instead of 128/256 MB). Output is
produced in 8 column passes; each pass accumulates all 1024 output rows
x 512 cols in the 8 PSUM banks over the 64 k-chunks. DMA issue is
spread over three queues (A loads on scalar, B loads on sync, stores on
gpsimd) so no single sequencer serializes the stream.
"""

import os
from contextlib import ExitStack

import numpy as np

WS = 8
M = 8192
N = 4096
LK = 1024
K = WS * LK          # 8192 contraction (rank*local_k)
M_LOC = M // WS      # 1024 output rows per core
N_CORES = 8

# compute dtype: "bfloat16" (default; rel err ~2.3e-3), "float32r"
# (rel err ~1.5e-4, but B no longer fits a single streaming pass:
# falls back to the two-pass schedule), "float32" (exact, 4x slower)
DTYPE = os.environ.get("BASS_KERNEL_DTYPE", "bfloat16")

_NC_CACHE = {}


def _build_nc(dt_name):
    import concourse.tile as tile
    from concourse import bacc, mybir
    from concourse.bass import ds

    f32 = mybir.dt.float32
    if dt_name == "bfloat16":
        sb_dt = mybir.dt.bfloat16
    elif dt_name == "float32r":
        sb_dt = mybir.dt.float32r
    elif dt_name == "float32":
        sb_dt = f32
    else:
        raise ValueError(dt_name)
    esize = 2 if dt_name == "bfloat16" else 4

    KC = K // 128                     # 64 k-chunks
    MS = M_LOC // 128                 # 8 m-subtiles (psum partition blocks)
    # matmul moving width (psum free cols per matmul). 512 = 1 psum bank
    # per m-subtile x 8 subtiles; 1024 = 2 banks x 4 subtiles (halves PE
    # instruction count, streams B twice).
    NBW = int(os.environ.get("BASS_NBW", "512"))
    PSUM_F32 = 4096                   # 8 banks x 512 f32 per partition
    NBLK = N // NBW                   # n-blocks
    # pack BPAIR k-chunks into each B DMA (3D access pattern): halves the
    # B DMA count and the PE-side DMA-completion waits
    BPAIR = int(os.environ.get("BASS_BPAIR", "1"))
    B_BUFS = int(os.environ.get("BASS_B_BUFS", str(24 // BPAIR)))
    # timing-only diagnostics (break correctness):
    FAKE_B = os.environ.get("BASS_FAKE_B", "0") == "1"
    NO_OUT = os.environ.get("BASS_NO_OUT", "0") == "1"
    # spread psum->sbuf copies across two engines
    COPY_SPREAD = os.environ.get("BASS_COPY_SPREAD", "0") == "1"
    # hardware-loop the n-block passes (shrinks the NEFF ~8x: the unrolled
    # per-pass body is stored once and re-executed with the column offset in
    # a register). Stationary (ldweights) addresses are pass-invariant, so
    # only the B-load source and store destination use the loop variable.
    FORI = os.environ.get("BASS_FORI", "0") == "1"
    # A-resident needs KC*[128,1024] tiles: 16MB bf16 (fits), 32MB f32 (not).
    # 4-byte dtypes keep a half-width [K, 512] block resident instead (16MB)
    # and stream B twice.
    A_RESIDENT = esize * K * M_LOC <= 20 * 2**20
    A_BUFS = KC
    REPS = int(os.environ.get("BASS_REPS", "1"))

    nc = bacc.Bacc("TRN2", target_bir_lowering=False, debug=False,
                   num_devices=N_CORES)
    a_d = nc.dram_tensor("a", [K, M_LOC], sb_dt, kind="ExternalInput")
    b_d = nc.dram_tensor("b", [K, N], sb_dt, kind="ExternalInput")
    o_d = nc.dram_tensor("o", [M_LOC, N], f32, kind="ExternalOutput")

    with tile.TileContext(nc) as tc, ExitStack() as ctx:
        apool = ctx.enter_context(tc.tile_pool(name="apool", bufs=A_BUFS))
        bpool = ctx.enter_context(tc.tile_pool(name="bpool", bufs=B_BUFS))
        cpool = ctx.enter_context(
            tc.tile_pool(name="cpool", bufs=8 if NBW <= 512 else 4))
        pp = ctx.enter_context(tc.tile_pool(name="pp", bufs=8, space="PSUM"))

        def load_a(kc, half=None):
            # half=None loads all M_LOC cols; 0/1 load 512-col halves
            # (two-pass fallback for 4-byte dtypes)
            c0 = 0 if half in (None, 0) else 512
            cw = M_LOC if half is None else 512
            a_t = apool.tile([128, cw], sb_dt, name=f"a_{kc}", tag="a")
            nc.scalar.dma_start(
                a_t[:], a_d.ap()[kc * 128:(kc + 1) * 128, c0:c0 + cw])
            return a_t

        for rep in range(REPS):
            if A_RESIDENT:
                n_ms = PSUM_F32 // NBW    # m-subtiles per psum group
                mblocks = [(j * n_ms, (j + 1) * n_ms)
                           for j in range(MS // n_ms)]
            else:
                mblocks = [(0, MS // 2), (MS // 2, MS)]
            for mb, (ms_lo, ms_hi) in enumerate(mblocks):
                a_tiles = [load_a(kc, None if A_RESIDENT else mb)
                           for kc in range(KC)]
                n_ms = ms_hi - ms_lo

                def pass_body(nb, col0, skip_out=False):
                    # col0: n-column offset of this pass (int, or the For_i
                    # loop register). nb is only used for tile naming.
                    psums = [pp.tile([128, NBW], f32, name=f"p_{nb}_{ms}",
                                     tag=f"p{ms}", bufs=1)
                             for ms in range(n_ms)]
                    for kc0 in range(0, KC, BPAIR):
                        if FAKE_B:
                            b_t = fake_b
                        elif BPAIR == 1:
                            b_t = bpool.tile([128, NBW], sb_dt,
                                             name=f"b_{nb}_{kc0}", tag="b")
                            nc.sync.dma_start(
                                b_t[:],
                                b_d.ap()[kc0 * 128:(kc0 + 1) * 128,
                                         ds(col0, NBW)])
                        else:
                            b_t = bpool.tile([128, BPAIR * NBW], sb_dt,
                                             name=f"b_{nb}_{kc0}", tag="b")
                            src = b_d.ap()[kc0 * 128:(kc0 + BPAIR) * 128,
                                           ds(col0, NBW)]
                            nc.sync.dma_start(
                                b_t[:].rearrange("p (j c) -> p j c", j=BPAIR),
                                src.rearrange("(j p) c -> p j c", p=128))
                        for j in range(BPAIR if not FAKE_B else BPAIR):
                            kc = kc0 + j
                            bt_v = (b_t[:] if (FAKE_B or BPAIR == 1) else
                                    b_t[:, j * NBW:(j + 1) * NBW])
                            at = a_tiles[kc]
                            for i in range(n_ms):
                                ci = (ms_lo + i) if A_RESIDENT else i
                                nc.tensor.matmul(
                                    psums[i][:],
                                    at[:, ci * 128:(ci + 1) * 128],
                                    bt_v,
                                    start=(kc == 0),
                                    stop=(kc == KC - 1))
                    if skip_out:
                        return
                    for i in range(n_ms):
                        ms = ms_lo + i
                        c_t = cpool.tile([128, NBW], f32,
                                         name=f"c_{nb}_{ms}", tag="c")
                        if COPY_SPREAD and i % 2:
                            nc.scalar.copy(c_t[:], psums[i][:])
                        else:
                            nc.vector.tensor_copy(c_t[:], psums[i][:])
                        nc.gpsimd.dma_start(
                            o_d.ap()[ms * 128:(ms + 1) * 128,
                                     ds(col0, NBW)],
                            c_t[:])

                if FAKE_B:
                    fake_b = bpool.tile([128, NBW], sb_dt,
                                        name=f"b_fake_{rep}_{mb}", tag="b")
                    nc.sync.dma_start(fake_b[:], b_d.ap()[0:128, 0:NBW])
                if FORI:
                    # FORI_UNROLL passes per hw-loop iteration: fewer
                    # iteration barriers at the cost of a bigger loop body
                    FORI_UNROLL = int(os.environ.get("BASS_FORI_UNROLL", "1"))
                    STAG = os.environ.get("BASS_FORI_STAG", "0") == "1"
                    with tc.For_i(0, N, NBW * FORI_UNROLL,
                                  staggered_reset=STAG) as col0:
                        for u in range(FORI_UNROLL):
                            pass_body(f"i{u}", col0 + u * NBW)
                else:
                    for nb in range(NBLK):
                        last = (nb == NBLK - 1 and rep == REPS - 1
                                and mb == len(mblocks) - 1)
                        pass_body(nb, nb * NBW,
                                  skip_out=NO_OUT and not last)

    nc.compile()
    return nc


def get_nc(dt_name=None):
    dt_name = dt_name or DTYPE
    if dt_name not in _NC_CACHE:
        _NC_CACHE[dt_name] = _build_nc(dt_name)
    return _NC_CACHE[dt_name]


def make_in_maps(input, weight, dt_name=None):
    """Host-side shard + layout prep. Returns in_maps for cores 0..7."""
    dt_name = dt_name or DTYPE
    input = np.asarray(input, dtype=np.float32)
    weight = np.asarray(weight, dtype=np.float32)
    assert input.shape == (WS, M, LK), input.shape
    assert weight.shape == (WS, N, LK), weight.shape

    if dt_name == "bfloat16":
        import ml_dtypes
        np_dt = ml_dtypes.bfloat16
    else:
        np_dt = np.float32

    # B[r*LK + k, n] = weight[r, n, k]  -> [K, N]
    b_full = np.ascontiguousarray(
        weight.transpose(0, 2, 1).reshape(K, N).astype(np_dt))
    in_maps = []
    for c in range(N_CORES):
        # A_c[r*LK + k, m] = input[r, c*M_LOC + m, k]  -> [K, M_LOC]
        a_c = np.ascontiguousarray(
            input[:, c * M_LOC:(c + 1) * M_LOC, :]
            .transpose(0, 2, 1).reshape(K, M_LOC).astype(np_dt))
        in_maps.append({"a": a_c, "b": b_full})
    return in_maps


def kernel(input, weight):
    from concourse import bass_utils

    nc = get_nc()
    in_maps = make_in_maps(input, weight)
    res = bass_utils.run_bass_kernel_spmd(
        nc, in_maps, core_ids=list(range(N_CORES)))
    out = np.stack([res.results[c]["o"] for c in range(N_CORES)], axis=0)
    return out.astype(np.float32)


# revision 22
# speedup vs baseline: 1.5525x; 1.5525x over previous
"""GEMM + reduce-scatter (nn_GemmRSIntraNode) as a Bass/Tile kernel on 8 trn2 cores.

Full semantics: out = einsum('rmk,rnk->mn', input, weight).reshape(8, 1024, 4096)
with input [8, 8192, 1024] f32 and weight [8, 4096, 1024] f32.

Sharding choice: instead of mimicking the per-rank partial-GEMM +
reduce-scatter, each core c directly computes output rows
[c*1024:(c+1)*1024] of the reduced result:

    out_c = sum_{r,k} input[r, c*1024:(c+1)*1024, k] * weight[r, n, k]

i.e. a [1024, 8192] x [8192, 4096] GEMM per core where the contraction
axis is (r, k) flattened. The rank-sum IS the K-axis contraction, so no
cross-core communication is needed at all; the "reduce-scatter" is
absorbed into the GEMM. Inputs are pre-transposed host-side into
[K, M] / [K, N] layouts so the device kernel does only contiguous DMA
and matmuls.

Device schedule (v2): compute in bf16 (norm rel err ~2.3e-3, well under
the 2e-2 gate; same PE rate as fp32r but half the HBM traffic). The
whole A slice [K, 1024] bf16 = 16 MB stays resident in SBUF, so B
streams through exactly once (64 MB instead of 128/256 MB). Output is
produced in 8 column passes; each pass accumulates all 1024 output rows
x 512 cols in the 8 PSUM banks over the 64 k-chunks. DMA issue is
spread over three queues (A loads on scalar, B loads on sync, stores on
gpsimd) so no single sequencer serializes the stream.
"""

import os
from contextlib import ExitStack

import numpy as np

WS = 8
M = 8192
N = 4096
LK = 1024
K = WS * LK          # 8192 contraction (rank*local_k)
M_LOC = M // WS      # 1024 output rows per core
N_CORES = 8

# compute dtype: "bfloat16" (default; rel err ~2.3e-3), "float32r"
# (rel err ~1.5e-4, but B no longer fits a single streaming pass:
# falls back to the two-pass schedule), "float32" (exact, 4x slower)
DTYPE = os.environ.get("BASS_KERNEL_DTYPE", "bfloat16")

_NC_CACHE = {}


def _build_nc(dt_name):
    import concourse.tile as tile
    from concourse import bacc, mybir
    from concourse.bass import ds

    f32 = mybir.dt.float32
    if dt_name == "bfloat16":
        sb_dt = mybir.dt.bfloat16
    elif dt_name == "float32r":
        sb_dt = mybir.dt.float32r
    elif dt_name == "float32":
        sb_dt = f32
    else:
        raise ValueError(dt_name)
    esize = 2 if dt_name == "bfloat16" else 4

    KC = K // 128                     # 64 k-chunks
    MS = M_LOC // 128                 # 8 m-subtiles (psum partition blocks)
    # matmul moving width (psum free cols per matmul). 512 = 1 psum bank
    # per m-subtile x 8 subtiles; 1024 = 2 banks x 4 subtiles (halves PE
    # instruction count, streams B twice).
    NBW = int(os.environ.get("BASS_NBW", "512"))
    PSUM_F32 = 4096                   # 8 banks x 512 f32 per partition
    NBLK = N // NBW                   # n-blocks
    # pack BPAIR k-chunks into each B DMA (3D access pattern). Measured
    # slightly slower than single-chunk DMAs on hw (strided descriptors),
    # so default 1.
    BPAIR = int(os.environ.get("BASS_BPAIR", "1"))
    B_BUFS = int(os.environ.get("BASS_B_BUFS", str(24 // BPAIR)))
    # timing-only diagnostics (break correctness):
    FAKE_B = os.environ.get("BASS_FAKE_B", "0") == "1"
    NO_OUT = os.environ.get("BASS_NO_OUT", "0") == "1"
    # spread psum->sbuf copies across two engines
    COPY_SPREAD = os.environ.get("BASS_COPY_SPREAD", "0") == "1"
    # hardware-loop the n-block passes (shrinks the NEFF ~8x: the unrolled
    # per-pass body is stored once and re-executed with the column offset in
    # a register). Stationary (ldweights) addresses are pass-invariant, so
    # only the B-load source and store destination use the loop variable.
    FORI = os.environ.get("BASS_FORI", "0") == "1"
    # A-resident needs KC*[128,1024] tiles: 16MB bf16 (fits), 32MB f32 (not).
    # 4-byte dtypes keep a half-width [K, 512] block resident instead (16MB)
    # and stream B twice.
    A_RESIDENT = esize * K * M_LOC <= 20 * 2**20
    A_BUFS = KC
    REPS = int(os.environ.get("BASS_REPS", "1"))

    nc = bacc.Bacc("TRN2", target_bir_lowering=False, debug=False,
                   num_devices=N_CORES)
    a_d = nc.dram_tensor("a", [K, M_LOC], sb_dt, kind="ExternalInput")
    b_d = nc.dram_tensor("b", [K, N], sb_dt, kind="ExternalInput")
    o_d = nc.dram_tensor("o", [M_LOC, N], f32, kind="ExternalOutput")

    with tile.TileContext(nc) as tc, ExitStack() as ctx:
        apool = ctx.enter_context(tc.tile_pool(name="apool", bufs=A_BUFS))
        bpool = ctx.enter_context(tc.tile_pool(name="bpool", bufs=B_BUFS))
        cpool = ctx.enter_context(
            tc.tile_pool(name="cpool", bufs=8 if NBW <= 512 else 4))
        pp = ctx.enter_context(tc.tile_pool(name="pp", bufs=8, space="PSUM"))

        def load_a(kc, half=None):
            # half=None loads all M_LOC cols; 0/1 load 512-col halves
            # (two-pass fallback for 4-byte dtypes)
            c0 = 0 if half in (None, 0) else 512
            cw = M_LOC if half is None else 512
            a_t = apool.tile([128, cw], sb_dt, name=f"a_{kc}", tag="a")
            nc.scalar.dma_start(
                a_t[:], a_d.ap()[kc * 128:(kc + 1) * 128, c0:c0 + cw])
            return a_t

        for rep in range(REPS):
            if A_RESIDENT:
                n_ms = PSUM_F32 // NBW    # m-subtiles per psum group
                mblocks = [(j * n_ms, (j + 1) * n_ms)
                           for j in range(MS // n_ms)]
            else:
                mblocks = [(0, MS // 2), (MS // 2, MS)]
            for mb, (ms_lo, ms_hi) in enumerate(mblocks):
                a_tiles = [load_a(kc, None if A_RESIDENT else mb)
                           for kc in range(KC)]
                n_ms = ms_hi - ms_lo

                def pass_body(nb, col0, skip_out=False):
                    # col0: n-column offset of this pass (int, or the For_i
                    # loop register). nb is only used for tile naming.
                    psums = [pp.tile([128, NBW], f32, name=f"p_{nb}_{ms}",
                                     tag=f"p{ms}", bufs=1)
                             for ms in range(n_ms)]
                    for kc0 in range(0, KC, BPAIR):
                        if FAKE_B:
                            b_t = fake_b
                        elif BPAIR == 1:
                            b_t = bpool.tile([128, NBW], sb_dt,
                                             name=f"b_{nb}_{kc0}", tag="b")
                            nc.sync.dma_start(
                                b_t[:],
                                b_d.ap()[kc0 * 128:(kc0 + 1) * 128,
                                         ds(col0, NBW)])
                        else:
                            b_t = bpool.tile([128, BPAIR * NBW], sb_dt,
                                             name=f"b_{nb}_{kc0}", tag="b")
                            src = b_d.ap()[kc0 * 128:(kc0 + BPAIR) * 128,
                                           ds(col0, NBW)]
                            nc.sync.dma_start(
                                b_t[:].rearrange("p (j c) -> p j c", j=BPAIR),
                                src.rearrange("(j p) c -> p j c", p=128))
                        for j in range(BPAIR if not FAKE_B else BPAIR):
                            kc = kc0 + j
                            bt_v = (b_t[:] if (FAKE_B or BPAIR == 1) else
                                    b_t[:, j * NBW:(j + 1) * NBW])
                            at = a_tiles[kc]
                            for i in range(n_ms):
                                ci = (ms_lo + i) if A_RESIDENT else i
                                nc.tensor.matmul(
                                    psums[i][:],
                                    at[:, ci * 128:(ci + 1) * 128],
                                    bt_v,
                                    start=(kc == 0),
                                    stop=(kc == KC - 1))
                    if skip_out:
                        return
                    for i in range(n_ms):
                        ms = ms_lo + i
                        c_t = cpool.tile([128, NBW], f32,
                                         name=f"c_{nb}_{ms}", tag="c")
                        if COPY_SPREAD and i % 2:
                            nc.scalar.copy(c_t[:], psums[i][:])
                        else:
                            nc.vector.tensor_copy(c_t[:], psums[i][:])
                        nc.gpsimd.dma_start(
                            o_d.ap()[ms * 128:(ms + 1) * 128,
                                     ds(col0, NBW)],
                            c_t[:])

                if FAKE_B:
                    fake_b = bpool.tile([128, NBW], sb_dt,
                                        name=f"b_fake_{rep}_{mb}", tag="b")
                    nc.sync.dma_start(fake_b[:], b_d.ap()[0:128, 0:NBW])
                if FORI:
                    # FORI_UNROLL passes per hw-loop iteration: fewer
                    # iteration barriers at the cost of a bigger loop body
                    FORI_UNROLL = int(os.environ.get("BASS_FORI_UNROLL", "1"))
                    STAG = os.environ.get("BASS_FORI_STAG", "0") == "1"
                    with tc.For_i(0, N, NBW * FORI_UNROLL,
                                  staggered_reset=STAG) as col0:
                        for u in range(FORI_UNROLL):
                            pass_body(f"i{u}", col0 + u * NBW)
                else:
                    for nb in range(NBLK):
                        last = (nb == NBLK - 1 and rep == REPS - 1
                                and mb == len(mblocks) - 1)
                        pass_body(nb, nb * NBW,
                                  skip_out=NO_OUT and not last)

    nc.compile()
    return nc


def get_nc(dt_name=None):
    dt_name = dt_name or DTYPE
    if dt_name not in _NC_CACHE:
        _NC_CACHE[dt_name] = _build_nc(dt_name)
    return _NC_CACHE[dt_name]


def make_in_maps(input, weight, dt_name=None):
    """Host-side shard + layout prep. Returns in_maps for cores 0..7."""
    dt_name = dt_name or DTYPE
    input = np.asarray(input, dtype=np.float32)
    weight = np.asarray(weight, dtype=np.float32)
    assert input.shape == (WS, M, LK), input.shape
    assert weight.shape == (WS, N, LK), weight.shape

    if dt_name == "bfloat16":
        import ml_dtypes
        np_dt = ml_dtypes.bfloat16
    else:
        np_dt = np.float32

    # B[r*LK + k, n] = weight[r, n, k]  -> [K, N]
    b_full = np.ascontiguousarray(
        weight.transpose(0, 2, 1).reshape(K, N).astype(np_dt))
    in_maps = []
    for c in range(N_CORES):
        # A_c[r*LK + k, m] = input[r, c*M_LOC + m, k]  -> [K, M_LOC]
        a_c = np.ascontiguousarray(
            input[:, c * M_LOC:(c + 1) * M_LOC, :]
            .transpose(0, 2, 1).reshape(K, M_LOC).astype(np_dt))
        in_maps.append({"a": a_c, "b": b_full})
    return in_maps


def kernel(input, weight):
    from concourse import bass_utils

    nc = get_nc()
    in_maps = make_in_maps(input, weight)
    res = bass_utils.run_bass_kernel_spmd(
        nc, in_maps, core_ids=list(range(N_CORES)))
    out = np.stack([res.results[c]["o"] for c in range(N_CORES)], axis=0)
    return out.astype(np.float32)
